# revision 21
# baseline (speedup 1.0000x reference)
"""Trainium2 Bass kernel for nn_CP_Based (CP-decomposition interaction layer).

Math (full problem):
    t[b,f,r,u] = sum_d X[b,f,d] * K[d,r,f,u]      (B=1024, F=64, D=4, R=32, U=128)
    had[b,r,u] = prod_f t[b,f,r,u]
    out[b,u]   = sum_r had[b,r,u]

Strategy (v2 — transposed layout, bf16 PE, 3-engine hadamard):
  * Feature-tripling (host-side repack): triple (f0,f1,f2) of features gives
    one K=64 contraction per factor; 22 factors total (21 triples + feature
    63 alone, zero-padded). Factor pairs share a 128-row PE pass via
    tile_position row tiling -> 11 groups.
  * Shard UNITS across 8 cores (U_loc=16): per-core kr is 1.4 MB and xt
    2.9 MB (both bf16) vs 23 MB fp32 replicated in v1 -> DMA ~13us.
  * TRANSPOSED layout: partitions = ru rows (u_loc*32+r), free dim = batch
    (1024).  4 ru-tiles of 128 rows; ops have 1024-elem free dims.
  * PE in bf16: 1 cyc/row (vs 4 for fp32) -> 45k cycles ~ 19us.
  * Hadamard split across engines (DVE tensor_tensor is 1x with a PSUM
    operand, 2x_1p for bf16 SBUF; ScalarE can copy PSUM->SBUF w/ downcast;
    GpSimd muls SBUF only at 0.42 eff):
      - DVE chain over 8 factors straight from PSUM (transit+mul in one op)
      - ScalarE copies 14 factors PSUM->SBUF bf16
      - DVE chains 8 of the copies at 2x_1p; GpSimd chains 6
      - final combine on DVE
  * r-sum for free on PE: out[u,b] = sel[ru,u].T @ P_total[ru,b] per ru-tile
    accumulated into one [16,1024] PSUM tile; host transposes per-core
    [16,1024] -> [1024,16] during gather.
"""

import numpy as np

B, F, D, R, U = 1024, 64, 4, 32, 128
NCORES = 8
ULOC = U // NCORES          # 16 units per core
RULOC = R * ULOC            # 512 ru rows per core
NTILE = RULOC // 128        # 4 ru-tiles of 128 partitions
NT = 21                     # feature triples
NFAC = 22                   # 21 triples + 1 padded single
NGRP = NFAC // 2            # 11 groups of 2 row-tiled factors
D3 = 64                     # contraction dim per triple (4^3)
NB = B                      # full batch on the free dim
MMN = 512                   # max moving free dim per matmul

# lane assignment: 'D' = DVE mul straight from PSUM, 'A' = ScalarE copy ->
# DVE fp16 chain, 'P' = ScalarE copy -> GpSimd chain.  Per ru-tile rt the
# table is rotated by SHIFT[rt] so that each (group, rt) unit of the
# interleaved emission mixes lanes across engines (phase-locked pipeline,
# no per-ru-tile boundary stalls).
LANE = [
    "D", "D", "A", "A", "A", "A", "D", "D", "A", "A", "A",
    "A", "D", "D", "P", "P", "P", "P", "D", "D", "P", "P",
]
SHIFT = [0, 5, 11, 16]

_cached = {}


def _build_nc():
    import concourse.bass as bass
    import concourse.mybir as mybir
    import concourse.tile as tile
    from concourse import bacc

    fp32 = mybir.dt.float32
    fp16 = mybir.dt.float16
    nc = bacc.Bacc("TRN2", target_bir_lowering=False, debug=False)

    xt_d = nc.dram_tensor("xt", [128, NGRP * NB], fp16, kind="ExternalInput").ap()
    kr_d = nc.dram_tensor("kr", [128, NGRP * RULOC], fp16, kind="ExternalInput").ap()
    sel_d = nc.dram_tensor("sel", [128, 4 * ULOC], fp16, kind="ExternalInput").ap()
    out_d = nc.dram_tensor("out", [ULOC, NB], fp32, kind="ExternalOutput").ap()

    with tile.TileContext(nc) as tc:
        with (
            tc.tile_pool(name="const", bufs=1) as cpool,
            tc.tile_pool(name="cf", bufs=10) as cfpool,
            tc.tile_pool(name="part", bufs=2) as ppool,
            tc.tile_pool(name="ps", bufs=3, space="PSUM") as pspool,
            tc.tile_pool(name="pso", bufs=1, space="PSUM") as opool,
        ):
            xt = cpool.tile([128, NGRP * NB], fp16)
            kr = cpool.tile([128, NGRP * RULOC], fp16)
            sel = cpool.tile([128, 4 * ULOC], fp16)
            # split input DMAs per group-pair so group m's matmuls only wait
            # on their own slice (a monolithic transfer stalls the PE ~7us)
            nc.sync.dma_start(kr[:, 0 : 2 * RULOC], kr_d[:, 0 : 2 * RULOC])
            nc.sync.dma_start(xt[:, 0 : 2 * NB], xt_d[:, 0 : 2 * NB])
            nc.sync.dma_start(sel[:], sel_d[:])
            for g in range(2, NGRP, 2):
                g2 = min(g + 2, NGRP)
                nc.sync.dma_start(
                    kr[:, g * RULOC : g2 * RULOC], kr_d[:, g * RULOC : g2 * RULOC]
                )
                nc.sync.dma_start(
                    xt[:, g * NB : g2 * NB], xt_d[:, g * NB : g2 * NB]
                )

            out_ps = opool.tile([ULOC, NB], fp32)

            # per-ru-tile chain state; all 4 ru-tile streams run interleaved
            # at group granularity so the engines stay phase-locked
            st = []
            for rt in range(NTILE):
                st.append(
                    dict(
                        P_dve=ppool.tile([128, NB], fp32, tag=f"pdve{rt}", name=f"pdve{rt}"),
                        P_act=ppool.tile([128, NB], fp16, tag=f"pact{rt}", name=f"pact{rt}"),
                        P_pool=ppool.tile([128, NB], fp16, tag=f"ppool{rt}", name=f"ppool{rt}"),
                        n_dve=0,
                        n_act=0,
                        n_pool=0,
                        first_act=None,
                        first_pool=None,
                    )
                )

            for m in range(NGRP):
                for rt in range(NTILE):
                    fac = []
                    for s in range(2):
                        pst = pspool.tile([128, NB], fp32, tag="ps")
                        for h in range(NB // MMN):
                            hs = slice(h * MMN, (h + 1) * MMN)
                            nc.tensor.matmul(
                                pst[:, hs],
                                kr[
                                    64 * s : 64 * s + D3,
                                    m * RULOC + 128 * rt : m * RULOC + 128 * rt + 128,
                                ],
                                xt[64 * s : 64 * s + D3, m * NB + h * MMN : m * NB + (h + 1) * MMN],
                                start=True,
                                stop=True,
                                tile_position=(64 * s, 0),
                            )
                        fac.append(pst)

                    S = st[rt]
                    for s in range(2):
                        j = 2 * m + s
                        lane = LANE[(j + SHIFT[rt]) % NFAC]
                        Fj = fac[s]
                        if lane == "D":
                            if S["n_dve"] == 0:
                                nc.vector.tensor_copy(S["P_dve"][:], Fj[:])
                            else:
                                nc.vector.tensor_mul(S["P_dve"][:], S["P_dve"][:], Fj[:])
                            S["n_dve"] += 1
                        else:
                            cf = cfpool.tile([128, NB], fp16, tag="cf")
                            nc.scalar.copy(cf[:], Fj[:])
                            if lane == "A":
                                if S["n_act"] == 0:
                                    S["first_act"] = cf
                                elif S["n_act"] == 1:
                                    nc.vector.tensor_mul(S["P_act"][:], S["first_act"][:], cf[:])
                                else:
                                    nc.vector.tensor_mul(S["P_act"][:], S["P_act"][:], cf[:])
                                S["n_act"] += 1
                            else:
                                if S["n_pool"] == 0:
                                    S["first_pool"] = cf
                                elif S["n_pool"] == 1:
                                    nc.gpsimd.tensor_mul(S["P_pool"][:], S["first_pool"][:], cf[:])
                                else:
                                    nc.gpsimd.tensor_mul(S["P_pool"][:], S["P_pool"][:], cf[:])
                                S["n_pool"] += 1

            for rt in range(NTILE):
                S = st[rt]
                X1 = ppool.tile([128, NB], fp16, tag=f"x1{rt}")
                P_tot = ppool.tile([128, NB], fp16, tag=f"ptot{rt}")
                nc.gpsimd.tensor_mul(X1[:], S["P_act"][:], S["P_pool"][:])
                nc.vector.tensor_mul(P_tot[:], S["P_dve"][:], X1[:])
                # accumulate all 4 ru-tiles into one [16, NB] psum tile; the
                # per-tile sel slice is nonzero only in columns 4rt..4rt+3
                for h in range(NB // MMN):
                    hs = slice(h * MMN, (h + 1) * MMN)
                    nc.tensor.matmul(
                        out_ps[:, hs],
                        sel[:, ULOC * rt : ULOC * rt + ULOC],
                        P_tot[:, hs],
                        start=(rt == 0),
                        stop=(rt == NTILE - 1),
                        skip_group_check=True,
                    )

            osb = cpool.tile([ULOC, NB], fp32)
            nc.scalar.copy(osb[:], out_ps[:])
            nc.sync.dma_start(out_d[:], osb[:])

    nc.compile()
    return nc


def _host_prep(X, K):
    """Repack inputs (all fp16):
      xt[row, m*NB + b]        : X3 outer products; row = 64*s + d3 holds
                                 factor j=2m+s; d3 = 16*d0+4*d1+d2.
      kr_c[row, m*RULOC + u_loc*32 + r] : K3 outer products, u-sliced per core.
      sel[k, t] = 1 if k//32 == t      : r-sum selection matrix.
    """
    f32 = np.float32

    xa = X[:, [3 * j for j in range(NT)], :]         # [B, 21, 4]
    xb = X[:, [3 * j + 1 for j in range(NT)], :]
    xc = X[:, [3 * j + 2 for j in range(NT)], :]
    X3 = (
        xa[:, :, :, None, None] * xb[:, :, None, :, None] * xc[:, :, None, None, :]
    ).reshape(B, NT, D3)                             # [B, 21, 64]
    X3f = np.zeros((B, NFAC, D3), dtype=f32)
    X3f[:, :NT] = X3
    X3f[:, NT, :D] = X[:, 63, :]
    # -> xt[row, m*NB+b]: [NFAC, D3, B] -> [NGRP, 2, D3, B] -> [128, NGRP*B]
    xt = (
        X3f.transpose(1, 2, 0)
        .reshape(NGRP, 2 * D3, B)
        .transpose(1, 0, 2)
        .reshape(2 * D3, NGRP * B)
    )
    xt = np.ascontiguousarray(xt).astype(np.float16)

    ka = K[:, :, [3 * j for j in range(NT)], :]      # [4, 32, 21, 128] (d,r,j,u)
    kb = K[:, :, [3 * j + 1 for j in range(NT)], :]
    kc = K[:, :, [3 * j + 2 for j in range(NT)], :]
    K3 = (
        ka[:, None, None] * kb[None, :, None] * kc[None, None, :]
    )                                                # [4,4,4,32,21,128] (d0,d1,d2,r,j,u)
    K3 = K3.transpose(4, 0, 1, 2, 3, 5).reshape(NT, D3, R, U)  # [j, d3, r, u]
    K3f = np.zeros((NFAC, D3, R, U), dtype=f32)
    K3f[:NT] = K3
    K3f[NT, :D] = K[:, :, 63, :]                     # lone feature 63
    krs = []
    for c in range(NCORES):
        Kc = K3f[:, :, :, c * ULOC : (c + 1) * ULOC]   # [NFAC, D3, R, ULOC]
        # cols ordered u_loc*32 + r  -> [NFAC, D3, ULOC, R]
        Kc = Kc.transpose(0, 1, 3, 2).reshape(NFAC, D3, RULOC)
        kr = (
            Kc.reshape(NGRP, 2, D3, RULOC)
            .transpose(1, 2, 0, 3)
            .reshape(2 * D3, NGRP * RULOC)
        )
        krs.append(np.ascontiguousarray(kr).astype(np.float16))

    selmat = np.zeros((128, 4 * ULOC), dtype=np.float16)
    for rt in range(NTILE):
        for k in range(128):
            selmat[k, ULOC * rt + 4 * rt + k // 32] = 1
    return xt, krs, selmat


def kernel(**inputs):
    from concourse.bass_utils import run_bass_kernel_spmd

    X = np.asarray(inputs["X"], dtype=np.float32)
    K = np.asarray(inputs["kernel"], dtype=np.float32)
    assert X.shape == (B, F, D) and K.shape == (D, R, F, U)

    if "nc" not in _cached:
        _cached["nc"] = _build_nc()
    nc = _cached["nc"]

    xt, krs, selmat = _host_prep(X, K)
    in_maps = [{"xt": xt, "kr": krs[c], "sel": selmat} for c in range(NCORES)]
    res = run_bass_kernel_spmd(nc, in_maps, core_ids=list(range(NCORES)))
    out = np.empty((B, U), dtype=np.float32)
    for c in range(NCORES):
        out[:, c * ULOC : (c + 1) * ULOC] = np.asarray(
            res.results[c]["out"], dtype=np.float32
        ).T
    return out


# revision 26
# speedup vs baseline: 1.1294x; 1.1294x over previous
"""Trainium2 Bass kernel for nn_CP_Based (CP-decomposition interaction layer).

Math (full problem):
    t[b,f,r,u] = sum_d X[b,f,d] * K[d,r,f,u]      (B=1024, F=64, D=4, R=32, U=128)
    had[b,r,u] = prod_f t[b,f,r,u]
    out[b,u]   = sum_r had[b,r,u]

Strategy (v2 — transposed layout, bf16 PE, 3-engine hadamard):
  * Feature-tripling (host-side repack): triple (f0,f1,f2) of features gives
    one K=64 contraction per factor; 22 factors total (21 triples + feature
    63 alone, zero-padded). Factor pairs share a 128-row PE pass via
    tile_position row tiling -> 11 groups.
  * Shard UNITS across 8 cores (U_loc=16): per-core kr is 1.4 MB and xt
    2.9 MB (both bf16) vs 23 MB fp32 replicated in v1 -> DMA ~13us.
  * TRANSPOSED layout: partitions = ru rows (u_loc*32+r), free dim = batch
    (1024).  4 ru-tiles of 128 rows; ops have 1024-elem free dims.
  * PE in bf16: 1 cyc/row (vs 4 for fp32) -> 45k cycles ~ 19us.
  * Hadamard split across engines (DVE tensor_tensor is 1x with a PSUM
    operand, 2x_1p for bf16 SBUF; ScalarE can copy PSUM->SBUF w/ downcast;
    GpSimd muls SBUF only at 0.42 eff):
      - DVE chain over 8 factors straight from PSUM (transit+mul in one op)
      - ScalarE copies 14 factors PSUM->SBUF bf16
      - DVE chains 8 of the copies at 2x_1p; GpSimd chains 6
      - final combine on DVE
  * r-sum for free on PE: out[u,b] = sel[ru,u].T @ P_total[ru,b] per ru-tile
    accumulated into one [16,1024] PSUM tile; host transposes per-core
    [16,1024] -> [1024,16] during gather.
"""

import numpy as np

B, F, D, R, U = 1024, 64, 4, 32, 128
NCORES = 8
ULOC = U // NCORES          # 16 units per core
RULOC = R * ULOC            # 512 ru rows per core
NTILE = RULOC // 128        # 4 ru-tiles of 128 partitions
NT = 21                     # feature triples
NFAC = 22                   # 21 triples + 1 padded single
NGRP = NFAC // 2            # 11 groups of 2 row-tiled factors
D3 = 64                     # contraction dim per triple (4^3)
NB = B                      # full batch on the free dim
MMN = 512                   # max moving free dim per matmul

# lane assignment: 'D' = DVE mul straight from PSUM, 'A' = ScalarE copy ->
# DVE fp16 chain, 'P' = ScalarE copy -> GpSimd chain.  Emission enumerates
# cells i = (m * NTILE + rt) * 2 + s; lane = MOTIF[i % 11].  The period-11
# motif is coprime with the 8-cell unit stride, so every ru-tile gets
# exactly 8 D / 8 A / 6 P while consecutive units keep a smooth lane mix
# (no all-Act or all-DVE units -> engines stay phase-locked).
MOTIF = ["D", "A", "P", "D", "A", "P", "D", "A", "P", "D", "A"]

_cached = {}


def _build_nc():
    import concourse.bass as bass
    import concourse.mybir as mybir
    import concourse.tile as tile
    from concourse import bacc

    fp32 = mybir.dt.float32
    fp16 = mybir.dt.float16
    nc = bacc.Bacc("TRN2", target_bir_lowering=False, debug=False)

    xt_d = nc.dram_tensor("xt", [128, NGRP * NB], fp16, kind="ExternalInput").ap()
    kr_d = nc.dram_tensor("kr", [128, NGRP * RULOC], fp16, kind="ExternalInput").ap()
    sel_d = nc.dram_tensor("sel", [128, 4 * ULOC], fp16, kind="ExternalInput").ap()
    out_d = nc.dram_tensor("out", [ULOC, NB], fp32, kind="ExternalOutput").ap()

    with tile.TileContext(nc) as tc:
        with (
            tc.tile_pool(name="const", bufs=1) as cpool,
            tc.tile_pool(name="cf", bufs=10) as cfpool,
            tc.tile_pool(name="part", bufs=2) as ppool,
            tc.tile_pool(name="ps", bufs=3, space="PSUM") as pspool,
            tc.tile_pool(name="pso", bufs=1, space="PSUM") as opool,
        ):
            xt = cpool.tile([128, NGRP * NB], fp16)
            kr = cpool.tile([128, NGRP * RULOC], fp16)
            sel = cpool.tile([128, 4 * ULOC], fp16)
            # split input DMAs per group-pair so group m's matmuls only wait
            # on their own slice (a monolithic transfer stalls the PE ~7us)
            nc.sync.dma_start(kr[:, 0:RULOC], kr_d[:, 0:RULOC])
            nc.sync.dma_start(xt[:, 0:NB], xt_d[:, 0:NB])
            nc.sync.dma_start(sel[:], sel_d[:])
            nc.sync.dma_start(kr[:, RULOC : 2 * RULOC], kr_d[:, RULOC : 2 * RULOC])
            nc.sync.dma_start(xt[:, NB : 2 * NB], xt_d[:, NB : 2 * NB])
            for g in range(2, NGRP, 2):
                g2 = min(g + 2, NGRP)
                nc.sync.dma_start(
                    kr[:, g * RULOC : g2 * RULOC], kr_d[:, g * RULOC : g2 * RULOC]
                )
                nc.sync.dma_start(
                    xt[:, g * NB : g2 * NB], xt_d[:, g * NB : g2 * NB]
                )

            out_ps = opool.tile([ULOC, NB], fp32)

            # per-ru-tile chain state; all 4 ru-tile streams run interleaved
            # at group granularity so the engines stay phase-locked
            st = []
            for rt in range(NTILE):
                st.append(
                    dict(
                        P_dve=ppool.tile([128, NB], fp32, tag=f"pdve{rt}", name=f"pdve{rt}"),
                        P_dvef=ppool.tile([128, NB], fp16, tag=f"pdvef{rt}", name=f"pdvef{rt}"),
                        P_act=ppool.tile([128, NB], fp16, tag=f"pact{rt}", name=f"pact{rt}"),
                        P_pool=ppool.tile([128, NB], fp16, tag=f"ppool{rt}", name=f"ppool{rt}"),
                        n_dve=0,
                        n_act=0,
                        n_pool=0,
                        first_act=None,
                        first_pool=None,
                    )
                )

            for m in range(NGRP):
                for rt in range(NTILE):
                    fac = []
                    for s in range(2):
                        pst = pspool.tile([128, NB], fp32, tag="ps")
                        for h in range(NB // MMN):
                            hs = slice(h * MMN, (h + 1) * MMN)
                            nc.tensor.matmul(
                                pst[:, hs],
                                kr[
                                    64 * s : 64 * s + D3,
                                    m * RULOC + 128 * rt : m * RULOC + 128 * rt + 128,
                                ],
                                xt[64 * s : 64 * s + D3, m * NB + h * MMN : m * NB + (h + 1) * MMN],
                                start=True,
                                stop=True,
                                tile_position=(64 * s, 0),
                            )
                        fac.append(pst)

                    S = st[rt]
                    for s in range(2):
                        j = 2 * m + s
                        lane = MOTIF[((m * NTILE + rt) * 2 + s) % 11]
                        Fj = fac[s]
                        if lane == "D":
                            if S["n_dve"] == 0:
                                nc.vector.tensor_copy(S["P_dve"][:], Fj[:])
                            elif S["n_dve"] == 7:
                                # last chain op downcasts so P_tot is a pure
                                # fp16 TT (mixed fp32xfp16 runs at half rate)
                                nc.vector.tensor_mul(S["P_dvef"][:], S["P_dve"][:], Fj[:])
                            else:
                                nc.vector.tensor_mul(S["P_dve"][:], S["P_dve"][:], Fj[:])
                            S["n_dve"] += 1
                        else:
                            cf = cfpool.tile([128, NB], fp16, tag="cf")
                            nc.scalar.copy(cf[:], Fj[:])
                            if lane == "A":
                                if S["n_act"] == 0:
                                    S["first_act"] = cf
                                elif S["n_act"] == 1:
                                    nc.vector.tensor_mul(S["P_act"][:], S["first_act"][:], cf[:])
                                else:
                                    nc.vector.tensor_mul(S["P_act"][:], S["P_act"][:], cf[:])
                                S["n_act"] += 1
                            else:
                                if S["n_pool"] == 0:
                                    S["first_pool"] = cf
                                elif S["n_pool"] == 1:
                                    nc.gpsimd.tensor_mul(S["P_pool"][:], S["first_pool"][:], cf[:])
                                else:
                                    nc.gpsimd.tensor_mul(S["P_pool"][:], S["P_pool"][:], cf[:])
                                S["n_pool"] += 1

            for rt in range(NTILE):
                S = st[rt]
                X1 = ppool.tile([128, NB], fp16, tag=f"x1{rt}")
                P_tot = ppool.tile([128, NB], fp16, tag=f"ptot{rt}")
                nc.gpsimd.tensor_mul(X1[:], S["P_act"][:], S["P_pool"][:])
                nc.vector.tensor_mul(P_tot[:], S["P_dvef"][:], X1[:])
                # accumulate all 4 ru-tiles into one [16, NB] psum tile; the
                # per-tile sel slice is nonzero only in columns 4rt..4rt+3
                for h in range(NB // MMN):
                    hs = slice(h * MMN, (h + 1) * MMN)
                    nc.tensor.matmul(
                        out_ps[:, hs],
                        sel[:, ULOC * rt : ULOC * rt + ULOC],
                        P_tot[:, hs],
                        start=(rt == 0),
                        stop=(rt == NTILE - 1),
                        skip_group_check=True,
                    )

            osb = cpool.tile([ULOC, NB], fp32)
            nc.scalar.copy(osb[:], out_ps[:])
            nc.sync.dma_start(out_d[:], osb[:])

    nc.compile()
    return nc


def _host_prep(X, K):
    """Repack inputs (all fp16):
      xt[row, m*NB + b]        : X3 outer products; row = 64*s + d3 holds
                                 factor j=2m+s; d3 = 16*d0+4*d1+d2.
      kr_c[row, m*RULOC + u_loc*32 + r] : K3 outer products, u-sliced per core.
      sel[k, t] = 1 if k//32 == t      : r-sum selection matrix.
    """
    f32 = np.float32

    xa = X[:, [3 * j for j in range(NT)], :]         # [B, 21, 4]
    xb = X[:, [3 * j + 1 for j in range(NT)], :]
    xc = X[:, [3 * j + 2 for j in range(NT)], :]
    X3 = (
        xa[:, :, :, None, None] * xb[:, :, None, :, None] * xc[:, :, None, None, :]
    ).reshape(B, NT, D3)                             # [B, 21, 64]
    X3f = np.zeros((B, NFAC, D3), dtype=f32)
    X3f[:, :NT] = X3
    X3f[:, NT, :D] = X[:, 63, :]
    # -> xt[row, m*NB+b]: [NFAC, D3, B] -> [NGRP, 2, D3, B] -> [128, NGRP*B]
    xt = (
        X3f.transpose(1, 2, 0)
        .reshape(NGRP, 2 * D3, B)
        .transpose(1, 0, 2)
        .reshape(2 * D3, NGRP * B)
    )
    xt = np.ascontiguousarray(xt).astype(np.float16)

    ka = K[:, :, [3 * j for j in range(NT)], :]      # [4, 32, 21, 128] (d,r,j,u)
    kb = K[:, :, [3 * j + 1 for j in range(NT)], :]
    kc = K[:, :, [3 * j + 2 for j in range(NT)], :]
    K3 = (
        ka[:, None, None] * kb[None, :, None] * kc[None, None, :]
    )                                                # [4,4,4,32,21,128] (d0,d1,d2,r,j,u)
    K3 = K3.transpose(4, 0, 1, 2, 3, 5).reshape(NT, D3, R, U)  # [j, d3, r, u]
    K3f = np.zeros((NFAC, D3, R, U), dtype=f32)
    K3f[:NT] = K3
    K3f[NT, :D] = K[:, :, 63, :]                     # lone feature 63
    krs = []
    for c in range(NCORES):
        Kc = K3f[:, :, :, c * ULOC : (c + 1) * ULOC]   # [NFAC, D3, R, ULOC]
        # cols ordered u_loc*32 + r  -> [NFAC, D3, ULOC, R]
        Kc = Kc.transpose(0, 1, 3, 2).reshape(NFAC, D3, RULOC)
        kr = (
            Kc.reshape(NGRP, 2, D3, RULOC)
            .transpose(1, 2, 0, 3)
            .reshape(2 * D3, NGRP * RULOC)
        )
        krs.append(np.ascontiguousarray(kr).astype(np.float16))

    selmat = np.zeros((128, 4 * ULOC), dtype=np.float16)
    for rt in range(NTILE):
        for k in range(128):
            selmat[k, ULOC * rt + 4 * rt + k // 32] = 1
    return xt, krs, selmat


def kernel(**inputs):
    from concourse.bass_utils import run_bass_kernel_spmd

    X = np.asarray(inputs["X"], dtype=np.float32)
    K = np.asarray(inputs["kernel"], dtype=np.float32)
    assert X.shape == (B, F, D) and K.shape == (D, R, F, U)

    if "nc" not in _cached:
        _cached["nc"] = _build_nc()
    nc = _cached["nc"]

    xt, krs, selmat = _host_prep(X, K)
    in_maps = [{"xt": xt, "kr": krs[c], "sel": selmat} for c in range(NCORES)]
    res = run_bass_kernel_spmd(nc, in_maps, core_ids=list(range(NCORES)))
    out = np.empty((B, U), dtype=np.float32)
    for c in range(NCORES):
        out[:, c * ULOC : (c + 1) * ULOC] = np.asarray(
            res.results[c]["out"], dtype=np.float32
        ).T
    return out
